# revision 67
# baseline (speedup 1.0000x reference)
"""Trainium2 Bass kernel for NarrativeClassificationLoss.

Data-parallel over batch: each of 8 cores processes a 2048-row shard and
emits per-class partial sums; the host combines them in float64 (the
pos_weight "all-reduce" over the batch happens at gather time, and the
per-class label sums feeding pos_weight are computed host-side from the
original int labels).

Host casts logits and labels to bf16 (labels are 0/1 -> exact) so the
device streams half the bytes and needs no on-device casts.

Per-element math (partition dim = batch rows, all intermediates bf16):
  s   = sigmoid(-x)                [ACT Sigmoid, scale=-1]
  L1  = ln(1-s)  = -softplus(-x)   [ACT Ln, scale=-1 bias=1]
  L2  = L1 - x   = -softplus(x)    [DVE]
  e   = s*s      = (1-sigmoid(x))^2 [DVE]
  u'  = y*L1     = -y*softplus(-x) [DVE]
  yspp' = y*L2   = -y*softplus(x)  [DVE]
Sigmoid and Ln live in different activation-table sets, so ACT work is
emitted in per-group phases (Sig-block then Ln-block over a few megatiles)
paying 2 table loads per group instead of 2 per megatile.

Per-class reductions over the batch go through TensorE into PSUM:
  C'[n,c]  = sum_b nl[b,n]*u'[b,c]      (= -C)
  Dp'[n,c] = sum_b nl[b,n]*L2[b,c]      (= -sum nl*spp)
  Dy'[n,c] = sum_b nl[b,n]*yspp'[b,c]   (= -sum nl*y*spp; D = Dy'-Dp')
  AB[0,:]  = ones-reduce of packed [u'_n | L2_n | yspp'_n]
  FC      += e_blk^T @ u'_blk  (+ 8x-scaled narrative part, so -tr(FC)
             carries both focal sums with the common 1/(B*1024) weight)
Hierarchy: sigmoid(max_k x_sub) = 1 - min_k s_sub, so the group reduction
is a pairwise MIN tree over s (DVE) and gd = s_n - min_s (Pool); masked
relu row-accumulate (DVE) finishes it.

Engine balance: e=s*s rides Pool for half the megas and DVE for the rest
(Pool can only copy/add/mult and cannot read PSUM); yspp likewise split.
u' overwrites the dead L1 slab and e overwrites the dead xs slab ("EX",
"UP" in INPLACE) - other aliasing combinations race on real hardware and
are left off. PSUM evacuation runs on ACT (idle at the tail).
"""

import numpy as np
import ml_dtypes

import concourse.bacc as bacc
import concourse.tile as tile
from concourse import mybir
from concourse.bass_utils import run_bass_kernel_spmd

B = 16384
NCORES = 8
BL = B // NCORES          # 2048 rows per core
NN = 128                  # narrative classes
NS = 1024                 # subnarrative classes
K = NS // NN              # 8 subnarratives per narrative
NT = BL // 128            # 16 batch tiles of 128 rows
MNT = 2                   # tiles per megatile
NMEGA = NT // MNT         # 8 megatiles
GROUPS = [4, 4]           # megas per activation-table phase group
LN_SPAN = 1               # megas covered by one wide Ln op (99 = whole group)
L1_BUFS = 4               # stream bufs for per-mega L1 (when LN_SPAN == 1)
E_POOL = {0, 1, 2, 3}     # megas whose e=s*s runs on Pool (else DVE)
E_ACT = {5, 6, 7}         # megas whose e runs on ACT as Square (table-free)
E8_ENGINE = "act"         # "act" | "pool" | "dve" for narr e8 = 8*s_n^2
E8_POS = "sig"            # "sig" | "ln0" | "ln_end": emission slot of e8
NARR_FC_LAST = False      # emit narr FC matmuls after all sub FC (chain tail)
YSPP_POOL = {4}           # megas whose yspp product runs on Pool (else DVE)
UP_POOL = set()           # megas whose u' product runs on Pool (else DVE)
L2_POOL = set()           # megas whose L2 subtract runs on Pool (else DVE)
NARR_GROUP = 0            # group index hosting the narrative chain
EVAC_DVE = {"C"}          # outputs whose PSUM evac runs on DVE (rest ACT)
DMA_ORDER = "a"           # input DMA issue order variant
SPLIT_HEAD = 1            # first N megas get per-tile DMA+Sig (earlier start)
SQ_DEFER = False          # emit E_ACT Squares after the group's Ln block
HALF_HEAD = False         # split tile 0's DMA+Sig into column halves
HIER_SPLIT = False        # hierarchy per mega (after its tree) vs per group
SQRT8 = 2.8284271247461903

f32 = mybir.dt.float32
bf16 = mybir.dt.bfloat16
AF = mybir.ActivationFunctionType
OP = mybir.AluOpType
BF = ml_dtypes.bfloat16

_CACHE = {}
LAST_RESULT = None


def _build(reps=1):
    nc = bacc.Bacc()

    xn = nc.declare_dram_parameter("narrative_logits", [BL, NN], bf16, isOutput=False)
    xs = nc.declare_dram_parameter("subnarrative_logits", [BL, NS], bf16, isOutput=False)
    yn = nc.declare_dram_parameter("narrative_labels", [BL, NN], bf16, isOutput=False)
    ys = nc.declare_dram_parameter("subnarrative_labels", [BL, NS], bf16, isOutput=False)

    o_c = nc.declare_dram_parameter("o_c", [NN, NS], bf16, isOutput=True)
    o_dp = nc.declare_dram_parameter("o_dp", [NN, NS], bf16, isOutput=True)
    o_dy = nc.declare_dram_parameter("o_dy", [NN, NS], bf16, isOutput=True)
    o_f = nc.declare_dram_parameter("o_f", [NN, NN], f32, isOutput=True)
    o_ab = nc.declare_dram_parameter("o_ab", [1, 384], f32, isOutput=True)
    o_h = nc.declare_dram_parameter("o_h", [NN, 8], f32, isOutput=True)

    with tile.TileContext(nc) as tc:
        with (
            tc.tile_pool(name="persist", bufs=1) as P1,
            tc.tile_pool(name="stream", bufs=2) as ST,
            tc.tile_pool(name="psum", bufs=1, space="PSUM") as PS,
        ):
            ones = P1.tile([128, 1], bf16)
            nc.vector.memset(ones, 1.0)
            for _rep in range(reps):
                _emit(nc, P1, ST, PS, ones, xn, xs, yn, ys,
                      o_c, o_dp, o_dy, o_f, o_ab, o_h)

    nc.finalize()
    return nc


def _emit(nc, P1, ST, PS, ones, xn, xs, yn, ys, o_c, o_dp, o_dy, o_f, o_ab, o_h):
    # ---------------- persistent slabs ----------------
    xs_all = P1.tile([128, NT, NS], bf16)     # sub logits
    s_all = P1.tile([128, NT, NS], bf16)      # sigmoid(-x_sub)
    ys_all = P1.tile([128, NT, NS], bf16)     # sub labels
    sm_all = P1.tile([128, NT, NN], bf16)     # group-min of s
    xn_b = P1.tile([128, NT, NN], bf16)
    yn_b = P1.tile([128, NT, NN], bf16)       # narrative labels (lhsT)
    s_n = P1.tile([128, NT, NN], bf16)
    L1_n = P1.tile([128, NT, NN], bf16)
    e8_n = P1.tile([128, NT, NN], bf16)
    L1_all = P1.tile([128, NT, NS], bf16)     # -softplus(-x_sub)
    ubn = P1.tile([128, NT, 384], bf16)       # [u'_n | L2_n | yspp'_n]
    u_n = ubn[:, :, 0:128]
    L2_n = ubn[:, :, 128:256]
    yspp_n = ubn[:, :, 256:384]
    hacc = P1.tile([128, 8], f32)
    nc.vector.memset(hacc, 0.0)

    # ---------------- PSUM accumulators (8 banks) ----------------
    C0 = PS.tile([128, 512], f32, tag="C0")
    C1 = PS.tile([128, 512], f32, tag="C1")
    Dp0 = PS.tile([128, 512], f32, tag="Dp0")
    Dp1 = PS.tile([128, 512], f32, tag="Dp1")
    Dy0 = PS.tile([128, 512], f32, tag="Dy0")
    Dy1 = PS.tile([128, 512], f32, tag="Dy1")
    FC = PS.tile([128, 128], f32, tag="FC")
    AB = PS.tile([1, 384], f32, tag="AB")

    xsr = xs[:, :].rearrange("(q p) c -> p q c", p=128)   # [128,16,1024]
    ysr = ys[:, :].rearrange("(q p) c -> p q c", p=128)
    xnr = xn[:, :].rearrange("(q p) c -> p q c", p=128)   # [128,16,128]
    ynr = yn[:, :].rearrange("(q p) c -> p q c", p=128)

    # ---------------- DMAs (SP queue order) ----------------
    # xs megas feed the Sig phases and must stay ahead of ACT; ys megas are
    # only needed at each group's Ln phase, xn/yn early for the narr chain.
    def dma_xs(m):
        t0 = m * MNT
        nc.sync.dma_start(out=xs_all[:, t0 : t0 + MNT, :],
                          in_=xsr[:, t0 : t0 + MNT, :])

    def dma_ys(m):
        t0 = m * MNT
        nc.sync.dma_start(out=ys_all[:, t0 : t0 + MNT, :],
                          in_=ysr[:, t0 : t0 + MNT, :])

    if DMA_ORDER == "a":
        if SPLIT_HEAD:
            if HALF_HEAD:
                nc.sync.dma_start(out=xs_all[:, 0:1, 0:512],
                                  in_=xsr[:, 0:1, 0:512])
                nc.sync.dma_start(out=xs_all[:, 0:1, 512:1024],
                                  in_=xsr[:, 0:1, 512:1024])
            else:
                nc.sync.dma_start(out=xs_all[:, 0:1, :], in_=xsr[:, 0:1, :])
            for t in range(1, 2 * SPLIT_HEAD):
                nc.sync.dma_start(out=xs_all[:, t : t + 1, :],
                                  in_=xsr[:, t : t + 1, :])
        for m in range(SPLIT_HEAD, 3):
            dma_xs(m)
        nc.sync.dma_start(out=xn_b, in_=xnr)
        dma_xs(3); dma_xs(4); dma_xs(5); dma_ys(0)
        nc.sync.dma_start(out=yn_b, in_=ynr)
        dma_xs(6); dma_ys(1); dma_xs(7)
        for m in range(2, NMEGA):
            dma_ys(m)
    elif DMA_ORDER == "b":
        dma_xs(0)
        nc.sync.dma_start(out=xn_b, in_=xnr)
        dma_xs(1); dma_xs(2); dma_xs(3)
        nc.sync.dma_start(out=yn_b, in_=ynr)
        dma_ys(0); dma_xs(4); dma_ys(1); dma_xs(5); dma_ys(2)
        dma_xs(6); dma_ys(3); dma_xs(7)
        for m in range(4, NMEGA):
            dma_ys(m)
    else:
        dma_xs(0); dma_xs(1)
        nc.sync.dma_start(out=xn_b, in_=xnr)
        dma_xs(2); dma_xs(3); dma_ys(0)
        nc.sync.dma_start(out=yn_b, in_=ynr)
        dma_xs(4); dma_ys(1); dma_xs(5); dma_ys(2); dma_xs(6)
        dma_ys(3); dma_xs(7)
        for m in range(4, NMEGA):
            dma_ys(m)

    # ---------------- helpers ----------------
    def sig_phase(m):
        """Phase 1 of a mega: sigmoid + min-tree (DVE)."""
        t0 = m * MNT
        st = s_all[:, t0 : t0 + MNT, :]
        if m < SPLIT_HEAD:
            if m == 0 and HALF_HEAD:
                nc.scalar.activation(s_all[:, 0:1, 0:512],
                                     xs_all[:, 0:1, 0:512],
                                     AF.Sigmoid, scale=-1.0)
                nc.scalar.activation(s_all[:, 0:1, 512:1024],
                                     xs_all[:, 0:1, 512:1024],
                                     AF.Sigmoid, scale=-1.0)
            else:
                nc.scalar.activation(s_all[:, t0 : t0 + 1, :],
                                     xs_all[:, t0 : t0 + 1, :],
                                     AF.Sigmoid, scale=-1.0)
            nc.scalar.activation(s_all[:, t0 + 1 : t0 + 2, :],
                                 xs_all[:, t0 + 1 : t0 + 2, :],
                                 AF.Sigmoid, scale=-1.0)
        else:
            nc.scalar.activation(st, xs_all[:, t0 : t0 + MNT, :],
                                 AF.Sigmoid, scale=-1.0)
        sg = st.rearrange("p q (n k) -> p q n k", k=K)
        m1 = ST.tile([128, MNT, NN, 4], bf16, tag="m1", bufs=1)
        nc.vector.tensor_tensor(m1, sg[:, :, :, 0:4], sg[:, :, :, 4:8], op=OP.min)
        m2 = ST.tile([128, MNT, NN, 2], bf16, tag="m2", bufs=1)
        nc.vector.tensor_tensor(m2, m1[:, :, :, 0:2], m1[:, :, :, 2:4], op=OP.min)
        nc.vector.tensor_tensor(
            sm_all[:, t0 : t0 + MNT, :], m2[:, :, :, 0], m2[:, :, :, 1], op=OP.min
        )
        if HIER_SPLIT:
            hier_chunk(m, t0, t0 + MNT)

    def ln_group(megas):
        """Wide Ln ops spanning LN_SPAN megas."""
        for i in range(0, len(megas), LN_SPAN):
            span = megas[i : i + LN_SPAN]
            t0, t1 = span[0] * MNT, (span[-1] + 1) * MNT
            nc.scalar.activation(L1_all[:, t0:t1, :], s_all[:, t0:t1, :],
                                 AF.Ln, scale=-1.0, bias=1.0)

    def ln_phase(m):
        """Phase 2 of a mega: elementwise products + matmuls. All products
        overwrite the dead persistent slabs in place (xs <- L2, L1 <- u',
        ys <- yspp', s <- e): no stream buffers, no buffer-count stalls."""
        t0 = m * MNT
        st = s_all[:, t0 : t0 + MNT, :]
        L1 = L1_all[:, t0 : t0 + MNT, :]
        yt = ys_all[:, t0 : t0 + MNT, :]
        L2 = xs_all[:, t0 : t0 + MNT, :]
        eng_l2 = nc.gpsimd if m in L2_POOL else nc.vector
        eng_l2.tensor_sub(L2, L1, L2)          # L2 = L1 - x, in place over xs
        up = L1
        eng_u = nc.gpsimd if m in UP_POOL else nc.vector
        eng_u.tensor_mul(up, yt, L1)           # u' = y*L1, in place over L1
        yp = yt
        eng_y = nc.gpsimd if m in YSPP_POOL else nc.vector
        eng_y.tensor_mul(yp, yt, L2)           # yspp' = y*L2, in place over ys
        e = st
        eng_e = nc.gpsimd if m in E_POOL else nc.vector
        eng_e.tensor_mul(e, st, st)            # e = s*s, in place over s
        for q in range(MNT):
            t = t0 + q
            st_ = t == 0
            sp = t == NT - 1
            nlT = yn_b[:, t, :]
            nc.tensor.matmul(C0, nlT, up[:, q, 0:512], start=st_, stop=sp)
            nc.tensor.matmul(C1, nlT, up[:, q, 512:1024], start=st_, stop=sp)
            nc.tensor.matmul(Dp0, nlT, L2[:, q, 0:512], start=st_, stop=sp)
            nc.tensor.matmul(Dp1, nlT, L2[:, q, 512:1024], start=st_, stop=sp)
            nc.tensor.matmul(Dy0, nlT, yp[:, q, 0:512], start=st_, stop=sp)
            nc.tensor.matmul(Dy1, nlT, yp[:, q, 512:1024], start=st_, stop=sp)
            for j in range(K):
                nc.tensor.matmul(
                    FC,
                    e[:, q, j * 128 : (j + 1) * 128],
                    up[:, q, j * 128 : (j + 1) * 128],
                    start=(t == 0 and j == 0 and (NARR_GROUP != 0 or NARR_FC_LAST)),
                    stop=(t == NT - 1 and j == K - 1 and not NARR_FC_LAST),
                )

    def hier_chunk(ci, r0, r1):
        rn = r1 - r0
        gd = ST.tile([128, rn, NN], bf16, tag="gd", bufs=1)
        nc.gpsimd.tensor_sub(gd, s_n[:, r0:r1, :], sm_all[:, r0:r1, :])
        hm = ST.tile([128, rn, NN], bf16, tag="hm", bufs=1)
        nc.gpsimd.tensor_mul(hm, gd, yn_b[:, r0:r1, :])
        nc.vector.tensor_scalar(gd, hm, 0.0, 0.0, op0=OP.max, op1=OP.add,
                                accum_out=hacc[:, ci : ci + 1])

    # ---------------- grouped phase schedule ----------------
    sq_deferred = []
    g0 = 0
    for gi, gsz in enumerate(GROUPS):
        megas = list(range(g0, g0 + gsz))
        g0 += gsz
        # --- Sig block ---
        for m in megas:
            sig_phase(m)
        def emit_e8():
            if E8_ENGINE == "act":
                nc.scalar.activation(e8_n, s_n, AF.Square, scale=SQRT8)
            else:
                eng8 = nc.gpsimd if E8_ENGINE == "pool" else nc.vector
                s8 = ST.tile([128, NT, NN], bf16, tag="s8", bufs=1)
                eng8.tensor_scalar(s8, s_n, SQRT8, 0.0, op0=OP.mult, op1=OP.add)
                eng8.tensor_mul(e8_n, s8, s8)

        if gi == NARR_GROUP:
            nc.scalar.activation(s_n, xn_b, AF.Sigmoid, scale=-1.0)
            if E8_POS == "sig":
                emit_e8()
        # --- Ln block ---
        ln_group(megas)
        if gi == NARR_GROUP:
            nc.scalar.activation(L1_n, s_n, AF.Ln, scale=-1.0, bias=1.0)
            if E8_POS == "ln0":
                emit_e8()
            nc.vector.tensor_sub(L2_n, L1_n, xn_b)
            nc.vector.tensor_mul(u_n, yn_b, L1_n)
            nc.vector.tensor_mul(yspp_n, yn_b, L2_n)
            for t in range(NT):
                nc.tensor.matmul(AB, ones, ubn[:, t, :],
                                 start=(t == 0), stop=(t == NT - 1))
            if not NARR_FC_LAST:
                for t in range(NT):
                    nc.tensor.matmul(FC, e8_n[:, t, :], u_n[:, t, :],
                                     start=(t == 0 and NARR_GROUP == 0),
                                     stop=False)
        for m in megas:
            ln_phase(m)
        if gi == NARR_GROUP and E8_POS == "ln_end":
            emit_e8()
        # --- hierarchy for this group's tile range ---
        hier_chunk(gi, megas[0] * MNT, (megas[-1] + 1) * MNT)

    if NARR_FC_LAST:
        for t in range(NT):
            nc.tensor.matmul(FC, e8_n[:, t, :], u_n[:, t, :],
                             start=False, stop=(t == NT - 1))

    # ---------------- evacuate (ACT is idle at the tail; GPSIMD can't read
    # PSUM) ----------------
    def evac(dst, src_, name=""):
        if name in EVAC_DVE:
            nc.vector.tensor_copy(out=dst, in_=src_)
        else:
            nc.scalar.copy(dst, src_)

    C_sb = P1.tile([128, NS], bf16)
    evac(C_sb[:, 0:512], C0, "C")
    evac(C_sb[:, 512:1024], C1, "C")
    Dp_sb = P1.tile([128, NS], bf16)
    evac(Dp_sb[:, 0:512], Dp0, "Dp")
    evac(Dp_sb[:, 512:1024], Dp1, "Dp")
    Dy_sb = P1.tile([128, NS], bf16)
    evac(Dy_sb[:, 0:512], Dy0, "Dy")
    evac(Dy_sb[:, 512:1024], Dy1, "Dy")
    F_sb = P1.tile([128, NN], f32)
    evac(F_sb, FC, "F")
    AB_sb = P1.tile([1, 384], f32)
    evac(AB_sb, AB, "AB")

    nc.sync.dma_start(out=o_c[:, :], in_=C_sb)
    nc.sync.dma_start(out=o_dp[:, :], in_=Dp_sb)
    nc.sync.dma_start(out=o_dy[:, :], in_=Dy_sb)
    nc.sync.dma_start(out=o_f[:, :], in_=F_sb)
    nc.sync.dma_start(out=o_ab[:, :], in_=AB_sb)
    nc.sync.dma_start(out=o_h[:, :], in_=hacc)


def kernel(
    narrative_logits, subnarrative_logits, narrative_labels, subnarrative_labels
):
    global LAST_RESULT
    if "nc" not in _CACHE:
        _CACHE["nc"] = _build()
    nc = _CACHE["nc"]

    # host-side pos_weight sums (the per-class reduction over the full batch)
    Sn = narrative_labels.sum(0, dtype=np.int64).astype(np.float64)
    Ss = subnarrative_labels.sum(0, dtype=np.int64).astype(np.float64)

    xnb = np.asarray(narrative_logits).astype(BF)
    xsb = np.asarray(subnarrative_logits).astype(BF)
    ynb = np.asarray(narrative_labels).astype(BF)    # 0/1 -> exact
    ysb = np.asarray(subnarrative_labels).astype(BF)

    in_maps = []
    for i in range(NCORES):
        s = slice(i * BL, (i + 1) * BL)
        in_maps.append(
            {
                "narrative_logits": np.ascontiguousarray(xnb[s]),
                "subnarrative_logits": np.ascontiguousarray(xsb[s]),
                "narrative_labels": np.ascontiguousarray(ynb[s]),
                "subnarrative_labels": np.ascontiguousarray(ysb[s]),
            }
        )

    res = run_bass_kernel_spmd(nc, in_maps, list(range(NCORES)))
    LAST_RESULT = res

    # ------- host combine (the batch "all-reduce") -------
    Cp = np.zeros((NN, NS), np.float64)
    Dp = np.zeros((NN, NS), np.float64)
    Dy = np.zeros((NN, NS), np.float64)
    F = np.zeros((NN, NN), np.float64)
    ABv = np.zeros(384, np.float64)
    H = 0.0
    for r in res.results:
        Cp += r["o_c"].astype(np.float64)
        Dp += r["o_dp"].astype(np.float64)
        Dy += r["o_dy"].astype(np.float64)
        F += r["o_f"].astype(np.float64)
        ABv += r["o_ab"][0].astype(np.float64)
        H += r["o_h"].astype(np.float64).sum()

    Ap = ABv[0:128]       # = -sum_b u_n
    Bp = ABv[128:256]     # = -sum_b spp_n
    By = ABv[256:384]     # = -sum_b y*spp_n

    npw = np.clip((B - Sn) / (Sn + 1e-6), 1.0, 50.0)
    spw = np.clip((B - Ss) / (Ss + 1e-6), 1.0, 50.0)

    narrative_loss = (npw * (-Ap) + (By - Bp)).sum() / (B * NN)

    cc = np.arange(NS)
    Cd = -Cp[cc // K, cc]
    Dd = (Dy - Dp)[cc // K, cc]
    valid = Sn.sum()
    sub_loss = (spw * Cd + Dd).sum() / K / max(valid, 1.0) if valid > 0 else 0.0

    hier = H / B
    focal = -0.1 * np.trace(F) / (B * NS)

    total = narrative_loss + sub_loss + 0.5 * hier + focal
    return np.asarray(total, dtype=np.float32)
